# revision 1
# baseline (speedup 1.0000x reference)
"""Trainium2 Bass kernel for nn_Enhanced_transformer (dense transformer block).

Strategy
--------
Data-parallel: batch B=8 -> one batch element per NeuronCore (8 cores), no
collectives. Per core, everything runs in channel-major ("transposed") layout
[channel-part, token-free], which makes every GEMM contraction land on the
partition dim with zero runtime transposes:

  h^T = LN1(x)^T           stats via ones-matmul partition reduction
  x_v^T = v_wT @ h^T       (f32r)        -> spill to DRAM
  x_q   = h^T' @ qk_wT     (fp32, [n-part, q-free])
  energy= x_q' @ x_q       (fp32, PSUM-accumulated across chunks)
  A1    = energy @ t1_wT   (fp32)  + t1_b (free-bcast)  -> gelu
  att2  = t2_wT' @ A1      (fp32)  + t2_b (part bias)   -> softmax -> f32r
  t_out^T = att' @ x_v^T   (f32r);  x1^T = t_out^T + x^T
  h2^T  = LN2(x1)^T        -> spill;  x1 -> spill
  m     = gelu(m1_wT' @ h2^T + m1_b);  out = m2_wT' @ m + m2_b + x1^T

dtypes: attention-logits chain exact fp32 (4 cyc/row but tiny FLOPs);
all big GEMMs float32r (tf32-like, 1 cyc/row at free>=256 == bf16 speed).
Expected absmax error vs fp32 reference ~7e-3 (simulated).

Host side: per-core transposes of x / weights; output transposed back.
"""

import numpy as np

import concourse.bass as bass
import concourse.tile as tile
from concourse import bacc, mybir
from concourse import bass_utils

F32 = mybir.dt.float32
F32R = mybir.dt.float32r
AF = mybir.ActivationFunctionType
ALU = mybir.AluOpType
AX = mybir.AxisListType

B, N, P = 8, 4096, 1024
P4 = P // 4          # 256
EPS = 1e-5
CH = 512             # token chunk
NCH = N // CH        # 8
KP = P // 128        # 8 channel tiles
KQ = P4 // 128       # 2


def _build(apply_ln1_affine: bool, apply_ln2_affine: bool, loop_R: int = 1):
    nc = bacc.Bacc("TRN2", target_bir_lowering=False, debug=False)

    # ---- DRAM I/O ----
    xT_d = nc.dram_tensor("xT", [P, N], F32, kind="ExternalInput").ap()
    qk_wT_d = nc.dram_tensor("qk_wT", [P, P4], F32, kind="ExternalInput").ap()
    v_wT_d = nc.dram_tensor("v_wT", [P, P], F32R, kind="ExternalInput").ap()
    t1_wT_d = nc.dram_tensor("t1_wT", [P4, P], F32, kind="ExternalInput").ap()
    t2_wT_d = nc.dram_tensor("t2_wT", [P4, P], F32, kind="ExternalInput").ap()
    m1_wT_d = nc.dram_tensor("m1_wT", [P, P], F32R, kind="ExternalInput").ap()
    m2_wT_d = nc.dram_tensor("m2_wT", [P, P], F32R, kind="ExternalInput").ap()
    v_b_d = nc.dram_tensor("v_b", [P], F32, kind="ExternalInput").ap()
    t1_b_d = nc.dram_tensor("t1_b", [P], F32, kind="ExternalInput").ap()
    t2_b_d = nc.dram_tensor("t2_b", [P], F32, kind="ExternalInput").ap()
    m1_b_d = nc.dram_tensor("m1_b", [P], F32, kind="ExternalInput").ap()
    m2_b_d = nc.dram_tensor("m2_b", [P], F32, kind="ExternalInput").ap()
    ln_d = {}
    if apply_ln1_affine:
        ln_d["ln1_g"] = nc.dram_tensor("ln1_g", [P], F32, kind="ExternalInput").ap()
        ln_d["ln1_b"] = nc.dram_tensor("ln1_b", [P], F32, kind="ExternalInput").ap()
    if apply_ln2_affine:
        ln_d["ln2_g"] = nc.dram_tensor("ln2_g", [P], F32, kind="ExternalInput").ap()
        ln_d["ln2_b"] = nc.dram_tensor("ln2_b", [P], F32, kind="ExternalInput").ap()
    outT_d = nc.dram_tensor("outT", [P, N], F32, kind="ExternalOutput").ap()

    def part_bias_tiles(pool, dram_ap, name):
        """[P] dram vector -> list of KP [128,1] per-partition SBUF tiles."""
        tiles = []
        for t in range(KP):
            bt = pool.tile([128, 1], F32, tag=f"{name}{t}", name=f"{name}{t}")
            nc.scalar.dma_start(bt[:], dram_ap[t * 128 : (t + 1) * 128])
            tiles.append(bt)
        return tiles

    with tile.TileContext(nc) as tc:
        with (
            tc.tile_pool(name="dram", bufs=1, space="DRAM") as dram_pool,
            tc.tile_pool(name="consts", bufs=1) as consts,
        ):
            xv_sp = dram_pool.tile([P, N], F32R, name="xv_sp")
            h2_sp = dram_pool.tile([P, N], F32R, name="h2_sp")
            x1_sp = dram_pool.tile([P, N], F32, name="x1_sp")
            att_sp = dram_pool.tile([P, P], F32R, name="att_sp")

            ones_f = consts.tile([128, 128], F32, tag="ones_f", name="ones_f")
            nc.vector.memset(ones_f[:], 1.0 / P)
            ones_r = consts.tile([128, 128], F32R, tag="ones_r", name="ones_r")
            nc.vector.tensor_copy(ones_r[:], ones_f[:])
            eps_t = consts.tile([128, 1], F32, tag="eps", name="eps_t")
            nc.vector.memset(eps_t[:], EPS)

            vb_t = part_bias_tiles(consts, v_b_d, "vb")
            t2b_t = part_bias_tiles(consts, t2_b_d, "t2b")
            m1b_t = part_bias_tiles(consts, m1_b_d, "m1b")
            m2b_t = part_bias_tiles(consts, m2_b_d, "m2b")
            # t1_b broadcast along partitions: [128, P] via stride-0 DMA
            t1b_bc = consts.tile([128, P], F32, tag="t1b_bc", name="t1b_bc")
            t1b_src = bass.AP(
                tensor=t1_b_d.tensor, offset=t1_b_d.offset,
                ap=[[0, 128], *t1_b_d.ap],
            )
            nc.scalar.dma_start(t1b_bc[:], t1b_src)
            ln_t = {}
            if apply_ln1_affine:
                ln_t["g1"] = part_bias_tiles(consts, ln_d["ln1_g"], "g1")
                ln_t["b1"] = part_bias_tiles(consts, ln_d["ln1_b"], "b1")
            if apply_ln2_affine:
                ln_t["g2"] = part_bias_tiles(consts, ln_d["ln2_g"], "g2")
                ln_t["b2"] = part_bias_tiles(consts, ln_d["ln2_b"], "b2")

            def ln_stats(psP, pP, x_tiles, sq_tiles, tag):
                """x_tiles: KP x [128,CH] fp32; sq_tiles f32r. -> (mu_b, rho_b)
                [128,CH] fp32, already broadcast to all partitions (full ones
                matrix in the matmul replicates the row-sums)."""
                ps_s = psP.tile([128, CH], F32, tag=f"{tag}_s", name=f"{tag}_s")
                ps_q = psP.tile([128, CH], F32, tag=f"{tag}_q", name=f"{tag}_q")
                for p in range(KP):
                    nc.tensor.matmul(
                        ps_s[:], ones_r[:], x_tiles[p][:],
                        start=(p == 0), stop=(p == KP - 1),
                    )
                for p in range(KP):
                    nc.tensor.matmul(
                        ps_q[:], ones_r[:], sq_tiles[p][:],
                        start=(p == 0), stop=(p == KP - 1),
                    )
                mu_b = pP.tile([128, CH], F32, tag=f"{tag}_mu", name=f"{tag}_mu")
                nc.vector.tensor_copy(mu_b[:], ps_s[:])
                var = pP.tile([128, CH], F32, tag=f"{tag}_var", name=f"{tag}_var")
                nc.vector.tensor_mul(var[:], mu_b[:], mu_b[:])
                nc.vector.tensor_tensor(var[:], ps_q[:], var[:], ALU.subtract)
                nc.scalar.activation(var[:], var[:], AF.Sqrt, bias=eps_t[:])
                rho_b = pP.tile([128, CH], F32, tag=f"{tag}_rho", name=f"{tag}_rho")
                nc.vector.reciprocal(rho_b[:], var[:])
                return mu_b, rho_b

            # Optional hardware repeat-loop for timing (test.py only).
            from contextlib import ExitStack as _ES
            _loop_ctx = _ES()
            if loop_R > 1:
                _loop_ctx.enter_context(tc.For_i(0, loop_R, 1))
            # ============ PHASE A: LN1, x_v, x_q, energy ============
            with tc.tile_pool(name="psE", bufs=1, space="PSUM") as psE:
                e_ps = [psE.tile([128, P4], F32, tag=f"e{i}", name=f"e_ps{i}")
                        for i in range(KQ)]
                with (
                    tc.tile_pool(name="wA", bufs=1) as wA,
                    tc.tile_pool(name="pA", bufs=1) as pA,
                    tc.tile_pool(name="psA", bufs=1, space="PSUM") as psA,
                ):
                    v_w_r = []
                    for p in range(KP):
                        wr = wA.tile([128, P], F32R, tag=f"vw{p}", name=f"vw{p}")
                        nc.scalar.dma_start(wr[:], v_wT_d[p * 128 : (p + 1) * 128, :])
                        v_w_r.append(wr)
                    qk_w_t = []
                    for p in range(KP):
                        wt = wA.tile([128, P4], F32, tag=f"qkw{p}", name=f"qkw{p}")
                        nc.scalar.dma_start(wt[:], qk_wT_d[p * 128 : (p + 1) * 128, :])
                        qk_w_t.append(wt)

                    for c in range(NCH):
                        cs = slice(c * CH, (c + 1) * CH)
                        xt = []
                        for p in range(KP):
                            t = pA.tile([128, CH], F32, tag=f"xt{p}", name=f"xt{p}",
                                        bufs=2)
                            nc.sync.dma_start(t[:], xT_d[p * 128 : (p + 1) * 128, cs])
                            xt.append(t)
                        xr, sq = [], []
                        for p in range(KP):
                            r = pA.tile([128, CH], F32R, tag=f"xr{p}", name=f"xr{p}")
                            nc.gpsimd.tensor_copy(r[:], xt[p][:])
                            xr.append(r)
                            s = pA.tile([128, CH], F32R, tag=f"sq{p}", name=f"sq{p}")
                            nc.scalar.activation(s[:], xt[p][:], AF.Square)
                            sq.append(s)
                        mu_b, rho_b = ln_stats(psA, pA, xr, sq, "st1")

                        h32, h_r = [], []
                        for p in range(KP):
                            h = pA.tile([128, CH], F32, tag=f"h32{p}", name=f"h32{p}",
                                        bufs=2)
                            nc.vector.tensor_tensor(h[:], xt[p][:], mu_b[:],
                                                    ALU.subtract)
                            nc.vector.tensor_mul(h[:], h[:], rho_b[:])
                            if apply_ln1_affine:
                                nc.scalar.activation(
                                    h[:], h[:], AF.Identity,
                                    bias=ln_t["b1"][p][:], scale=ln_t["g1"][p][:],
                                )
                            hr = pA.tile([128, CH], F32R, tag=f"hr{p}", name=f"hr{p}",
                                         bufs=2)
                            nc.scalar.activation(hr[:], h[:], AF.Copy)
                            h32.append(h)
                            h_r.append(hr)

                        # x_q chunk + energy accumulation
                        for ns in range(CH // 128):
                            ps = psA.tile([128, P4], F32, tag="xq", name="xq_ps",
                                          bufs=2)
                            for p in range(KP):
                                nc.tensor.matmul(
                                    ps[:],
                                    h32[p][:, ns * 128 : (ns + 1) * 128],
                                    qk_w_t[p][:],
                                    start=(p == 0), stop=(p == KP - 1),
                                )
                            xq = pA.tile([128, P4], F32, tag="xqs", name="xqs",
                                         bufs=3)
                            nc.vector.tensor_copy(xq[:], ps[:])
                            first = c == 0 and ns == 0
                            last = c == NCH - 1 and ns == CH // 128 - 1
                            for qh in range(KQ):
                                nc.tensor.matmul(
                                    e_ps[qh][:],
                                    xq[:, qh * 128 : (qh + 1) * 128],
                                    xq[:],
                                    start=first, stop=last,
                                    skip_group_check=True,
                                )

                        # x_v^T chunk
                        for o in range(KP):
                            ps = psA.tile([128, CH], F32, tag="xv", name="xv_ps",
                                          bufs=2)
                            for p in range(KP):
                                nc.tensor.matmul(
                                    ps[:], v_w_r[p][:, o * 128 : (o + 1) * 128],
                                    h_r[p][:], start=(p == 0), stop=(p == KP - 1),
                                )
                            xv = pA.tile([128, CH], F32R, tag="xvs", name="xvs",
                                         bufs=2)
                            nc.scalar.activation(
                                xv[:], ps[:], AF.Identity, bias=vb_t[o][:]
                            )
                            nc.sync.dma_start(
                                xv_sp[o * 128 : (o + 1) * 128, cs], xv[:]
                            )

                # ============ PHASE B: logits + softmax ============
                with (
                    tc.tile_pool(name="wB", bufs=1) as wB,
                    tc.tile_pool(name="pB", bufs=1) as pB,
                    tc.tile_pool(name="psB", bufs=1, space="PSUM") as psB,
                ):
                    t1_w_t, t2_w_t = [], []
                    for qh in range(KQ):
                        wt = wB.tile([128, P], F32, tag=f"t1w{qh}", name=f"t1w{qh}")
                        nc.scalar.dma_start(wt[:],
                                            t1_wT_d[qh * 128 : (qh + 1) * 128, :])
                        t1_w_t.append(wt)
                        wt2 = wB.tile([128, P], F32, tag=f"t2w{qh}", name=f"t2w{qh}")
                        nc.scalar.dma_start(wt2[:],
                                            t2_wT_d[qh * 128 : (qh + 1) * 128, :])
                        t2_w_t.append(wt2)
                    energy_sb = []
                    for qh in range(KQ):
                        e = wB.tile([128, P4], F32, tag=f"esb{qh}", name=f"esb{qh}")
                        nc.vector.tensor_copy(e[:], e_ps[qh][:])
                        energy_sb.append(e)

                    # A1[b, a] = sum_q energy[q,b] t1_wT[q,a]; +t1_b[a]; gelu
                    a1g = []
                    for bh in range(KQ):
                        a1 = pB.tile([128, P], F32, tag=f"a1_{bh}", name=f"a1_{bh}")
                        for oc in range(P // 512):
                            ps = psB.tile([128, 512], F32, tag="a1", name="a1_ps",
                                          bufs=2)
                            for qh in range(KQ):
                                nc.tensor.matmul(
                                    ps[:],
                                    energy_sb[qh][:, bh * 128 : (bh + 1) * 128],
                                    t1_w_t[qh][:, oc * 512 : (oc + 1) * 512],
                                    start=(qh == 0), stop=(qh == KQ - 1),
                                )
                            nc.vector.tensor_tensor(
                                a1[:, oc * 512 : (oc + 1) * 512], ps[:],
                                t1b_bc[:, oc * 512 : (oc + 1) * 512], ALU.add,
                            )
                        ag = wB.tile([128, P], F32, tag=f"a1g{bh}", name=f"a1g{bh}")
                        nc.scalar.activation(ag[:], a1[:], AF.Gelu)
                        a1g.append(ag)

                    # att2 + softmax -> att_r (f32r)
                    for o in range(KP):
                        att2 = pB.tile([128, P], F32, tag="att2", name="att2",
                                       bufs=2)
                        for kc in range(P // 512):
                            ps = psB.tile([128, 512], F32, tag="a2", name="a2_ps",
                                          bufs=2)
                            for ph in range(KQ):
                                nc.tensor.matmul(
                                    ps[:],
                                    t2_w_t[ph][:, o * 128 : (o + 1) * 128],
                                    a1g[ph][:, kc * 512 : (kc + 1) * 512],
                                    start=(ph == 0), stop=(ph == KQ - 1),
                                )
                            nc.scalar.activation(
                                att2[:, kc * 512 : (kc + 1) * 512], ps[:],
                                AF.Identity, bias=t2b_t[o][:],
                            )
                        negmax = pB.tile([128, 1], F32, tag="negmax", name="negmax",
                                         bufs=2)
                        nc.vector.tensor_reduce(
                            negmax[:], att2[:], axis=AX.X, op=ALU.max, negate=True
                        )
                        esum = pB.tile([128, 1], F32, tag="esum", name="esum",
                                       bufs=2)
                        expv = pB.tile([128, P], F32, tag="expv", name="expv",
                                       bufs=2)
                        nc.scalar.activation(
                            expv[:], att2[:], AF.Exp, bias=negmax[:],
                            accum_out=esum[:],
                        )
                        rec = pB.tile([128, 1], F32, tag="rec", name="rec", bufs=2)
                        nc.vector.reciprocal(rec[:], esum[:])
                        ar = pB.tile([128, P], F32R, tag="att_t", name="att_t",
                                     bufs=2)
                        nc.vector.tensor_scalar_mul(ar[:], expv[:], rec[:])
                        nc.sync.dma_start(
                            att_sp[o * 128 : (o + 1) * 128, :], ar[:]
                        )

            # ============ PHASE C1: t_out, x1, LN2, h2 ============
            with (
                tc.tile_pool(name="pC1", bufs=1) as pC,
                tc.tile_pool(name="psC1", bufs=1, space="PSUM") as psC,
            ):
                att_r = []
                for p in range(KP):
                    ar = pC.tile([128, P], F32R, tag=f"att{p}", name=f"att{p}")
                    nc.scalar.dma_start(ar[:], att_sp[p * 128 : (p + 1) * 128, :])
                    att_r.append(ar)

                def c1_tout_q(c, q, xt, xv):
                    cs = slice(c * CH, (c + 1) * CH)
                    nb = 1
                    ps = psC.tile([128, CH], F32, tag="tout", name="tout_ps",
                                  bufs=4)
                    for p in range(KP):
                        nc.tensor.matmul(
                            ps[:],
                            att_r[p][:, q * 128 : (q + 1) * 128],
                            xv[p][:],
                            start=(p == 0), stop=(p == KP - 1),
                        )
                    x1 = pC.tile([128, CH], F32, tag=f"x1{q}", name=f"x1{q}",
                                 bufs=nb)
                    nc.vector.tensor_tensor(x1[:], ps[:], xt[q][:], ALU.add)
                    nc.sync.dma_start(x1_sp[q * 128 : (q + 1) * 128, cs], x1[:])
                    r = pC.tile([128, CH], F32R, tag=f"x1r{q}", name=f"x1r{q}",
                                bufs=nb)
                    nc.gpsimd.tensor_copy(r[:], x1[:])
                    sq = pC.tile([128, CH], F32R, tag=f"sq2{q}", name=f"sq2{q}",
                                 bufs=nb)
                    nc.scalar.activation(sq[:], x1[:], AF.Square)
                    return x1, r, sq

                def c1_stats_h2(c, x1f, x1r, sq2):
                    cs = slice(c * CH, (c + 1) * CH)
                    mu2, rho2 = ln_stats(psC, pC, x1r, sq2, "st2")
                    for p in range(KP):
                        nc.vector.tensor_tensor(x1f[p][:], x1f[p][:], mu2[:],
                                                ALU.subtract)
                        h2r = pC.tile([128, CH], F32R, tag=f"h2r{p}",
                                      name=f"h2r{p}", bufs=2)
                        nc.vector.tensor_mul(h2r[:], x1f[p][:], rho2[:])
                        if apply_ln2_affine:
                            nc.scalar.activation(
                                h2r[:], h2r[:], AF.Identity,
                                bias=ln_t["b2"][p][:], scale=ln_t["g2"][p][:],
                            )
                        nc.sync.dma_start(
                            h2_sp[p * 128 : (p + 1) * 128, cs], h2r[:]
                        )

                for c in range(NCH):
                    cs = slice(c * CH, (c + 1) * CH)
                    xt = []
                    for p in range(KP):
                        t = pC.tile([128, CH], F32, tag=f"xt{p}", name=f"xt{p}",
                                    bufs=2)
                        nc.sync.dma_start(t[:], xT_d[p * 128 : (p + 1) * 128, cs])
                        xt.append(t)
                    xv = []
                    for p in range(KP):
                        t = pC.tile([128, CH], F32R, tag=f"xv{p}", name=f"xv{p}",
                                    bufs=2)
                        nc.sync.dma_start(t[:],
                                          xv_sp[p * 128 : (p + 1) * 128, cs])
                        xv.append(t)

                    x1f, x1r, sq2 = [], [], []
                    for q in range(KP):
                        a, b_, d = c1_tout_q(c, q, xt, xv)
                        x1f.append(a); x1r.append(b_); sq2.append(d)
                    c1_stats_h2(c, x1f, x1r, sq2)

            # ============ PHASE C2: MLP + final residual ============
            with (
                tc.tile_pool(name="wC2", bufs=1) as wC,
                tc.tile_pool(name="pC2", bufs=1) as pC2,
                tc.tile_pool(name="psC2", bufs=1, space="PSUM") as psC2,
            ):
                m1_w_r, m2_w_r = [], []
                for p in range(KP):
                    wr = wC.tile([128, P], F32R, tag=f"m1w{p}", name=f"m1w{p}")
                    nc.scalar.dma_start(wr[:], m1_wT_d[p * 128 : (p + 1) * 128, :])
                    m1_w_r.append(wr)
                for p in range(KP):
                    wr = wC.tile([128, P], F32R, tag=f"m2w{p}", name=f"m2w{p}")
                    nc.sync.dma_start(wr[:], m2_wT_d[p * 128 : (p + 1) * 128, :])
                    m2_w_r.append(wr)

                def c2_m1(c):
                    cs = slice(c * CH, (c + 1) * CH)
                    h2 = []
                    for p in range(KP):
                        t = pC2.tile([128, CH], F32R, tag=f"h2{p}", name=f"h2{p}",
                                     bufs=2)
                        nc.sync.dma_start(t[:], h2_sp[p * 128 : (p + 1) * 128, cs])
                        h2.append(t)
                    mg = []
                    for j in range(KP):
                        ps = psC2.tile([128, CH], F32, tag="m1", name="m1_ps",
                                       bufs=3)
                        for p in range(KP):
                            nc.tensor.matmul(
                                ps[:],
                                m1_w_r[p][:, j * 128 : (j + 1) * 128],
                                h2[p][:],
                                start=(p == 0), stop=(p == KP - 1),
                            )
                        g = pC2.tile([128, CH], F32R, tag=f"mg{j}", name=f"mg{j}",
                                     bufs=2)
                        nc.scalar.activation(g[:], ps[:], AF.Gelu, bias=m1b_t[j][:])
                        mg.append(g)
                    return mg

                def c2_m2(c, mg):
                    cs = slice(c * CH, (c + 1) * CH)
                    for o in range(KP):
                        x1 = pC2.tile([128, CH], F32, tag="x1l", name="x1l", bufs=3)
                        nc.sync.dma_start(x1[:], x1_sp[o * 128 : (o + 1) * 128, cs])
                        ps = psC2.tile([128, CH], F32, tag="m2", name="m2_ps",
                                       bufs=2)
                        for j in range(KP):
                            nc.tensor.matmul(
                                ps[:],
                                m2_w_r[j][:, o * 128 : (o + 1) * 128],
                                mg[j][:],
                                start=(j == 0), stop=(j == KP - 1),
                            )
                        mo = pC2.tile([128, CH], F32, tag="mo", name="mo", bufs=3)
                        nc.vector.scalar_tensor_tensor(
                            mo[:], ps[:], m2b_t[o][:], x1[:],
                            op0=ALU.add, op1=ALU.add,
                        )
                        nc.sync.dma_start(outT_d[o * 128 : (o + 1) * 128, cs], mo[:])

                for c in range(NCH):
                    c2_m2(c, c2_m1(c))

            _loop_ctx.close()

    nc.compile()
    return nc


_CACHE = {}


def _get_nc(apply_ln1_affine, apply_ln2_affine, loop_R=1):
    key = (apply_ln1_affine, apply_ln2_affine, loop_R)
    if key not in _CACHE:
        _CACHE[key] = _build(apply_ln1_affine, apply_ln2_affine, loop_R)
    return _CACHE[key]


def _round_f32r(x):
    """Round fp32 -> tf32-like (10 explicit mantissa bits, RNE)."""
    u = np.ascontiguousarray(x, np.float32).view(np.uint32)
    shift = 13
    bias = np.uint32((1 << (shift - 1)) - 1)
    lsb = (u >> np.uint32(shift)) & np.uint32(1)
    u2 = (u + bias + lsb) & np.uint32(~((1 << shift) - 1) & 0xFFFFFFFF)
    return u2.view(np.float32)


def kernel(**inputs):
    return _kernel_impl(inputs, loop_R=1)


def _kernel_impl(inputs, loop_R=1):
    x = np.ascontiguousarray(np.asarray(inputs["x"], np.float32))
    assert x.shape == (B, N, P), x.shape

    ln1_g = np.asarray(inputs["ln1_g"], np.float32)
    ln1_b = np.asarray(inputs["ln1_b"], np.float32)
    ln2_g = np.asarray(inputs["ln2_g"], np.float32)
    ln2_b = np.asarray(inputs["ln2_b"], np.float32)
    aff1 = not (np.all(ln1_g == 1.0) and np.all(ln1_b == 0.0))
    aff2 = not (np.all(ln2_g == 1.0) and np.all(ln2_b == 0.0))

    nc = _get_nc(aff1, aff2, loop_R)

    base = {
        "qk_wT": np.ascontiguousarray(np.asarray(inputs["qk_w"], np.float32).T),
        "v_wT": _round_f32r(np.asarray(inputs["v_w"], np.float32).T),
        "t1_wT": np.ascontiguousarray(np.asarray(inputs["t1_w"], np.float32).T),
        "t2_wT": np.ascontiguousarray(np.asarray(inputs["t2_w"], np.float32).T),
        "m1_wT": _round_f32r(np.asarray(inputs["m1_w"], np.float32).T),
        "m2_wT": _round_f32r(np.asarray(inputs["m2_w"], np.float32).T),
        "v_b": np.ascontiguousarray(np.asarray(inputs["v_b"], np.float32)),
        "t1_b": np.ascontiguousarray(np.asarray(inputs["t1_b"], np.float32)),
        "t2_b": np.ascontiguousarray(np.asarray(inputs["t2_b"], np.float32)),
        "m1_b": np.ascontiguousarray(np.asarray(inputs["m1_b"], np.float32)),
        "m2_b": np.ascontiguousarray(np.asarray(inputs["m2_b"], np.float32)),
    }
    if aff1:
        base["ln1_g"] = np.ascontiguousarray(ln1_g)
        base["ln1_b"] = np.ascontiguousarray(ln1_b)
    if aff2:
        base["ln2_g"] = np.ascontiguousarray(ln2_g)
        base["ln2_b"] = np.ascontiguousarray(ln2_b)

    in_maps = []
    for b in range(B):
        m = dict(base)
        m["xT"] = np.ascontiguousarray(x[b].T)
        in_maps.append(m)

    res = bass_utils.run_bass_kernel_spmd(nc, in_maps, core_ids=list(range(B)))
    out = np.empty((B, N, P), np.float32)
    for b in range(B):
        out[b] = res.results[b]["outT"].T
    return out


if __name__ == "__main__":
    import sys
    import time

    sys.path.insert(0, "/root/problem")
    import reference as refmod

    inputs = {k: np.asarray(v) for k, v in refmod.setup_inputs().items()}
    t0 = time.time()
    got = kernel(**inputs)
    print(f"kernel() took {time.time() - t0:.1f}s (incl compile)")
    t0 = time.time()
    got = kernel(**inputs)
    print(f"kernel() 2nd call {time.time() - t0:.1f}s")
    exp = np.asarray(refmod.reference(**inputs))
    err = np.abs(got - exp)
    scale = np.abs(exp).max()
    l2 = np.linalg.norm(got - exp) / np.linalg.norm(exp)
    print(f"absmax={err.max():.3e} scale-rel={err.max() / scale:.3e} L2rel={l2:.3e}")



# revision 3
# speedup vs baseline: 3.1759x; 3.1759x over previous
"""Trainium2 Bass kernel for nn_Enhanced_transformer (dense transformer block).

Strategy (v2)
-------------
Data-parallel: one batch element per NeuronCore (8 cores), no collectives.
Channel-major layout [channel-part, token-free] throughout. All GEMMs run at
1 cyc/row (f32r or bf16); LN is fused as post-GEMM affine corrections so the
normalized tensor h is never materialized; the value path is algebraically
restructured:

    t_out^T = att^T @ x_v^T = (att^T v_w) @ h^T = W_eff @ h^T

which replaces the two big GEMMs (x_v then t_out, 2 x 262k cyc) with one
small build GEMM (65k) + one big apply GEMM (262k), and eliminates the x_v
[P,N] spill entirely. No DRAM activation spills at all: only x is (re)read
in phases A and C; per-token LN1 stats bounce through tiny DRAM rows to
switch between token-partition and broadcast-row layouts.

  A (per 512-token chunk):  sq=x^2 (ACT), msq = ones@sq (PE, broadcast),
    x_q = x^T' @ qk_ext (PE, token-part, mean rides as an extra column),
    rho/(-mu rho) in token layout (DVE), x_q corrected (DVE),
    energy += x_q' x_q (PE, PSUM accum across chunks, 1-chunk pipelined)
  B: a1^T = E @ t1_wT (+bias, gelu) -> att2 = t2_wT' @ G (+bias) -> softmax
     -> att (bf16);  bsum = att^T v_b, W_effT = v_w^T-stat @ att,
     wsum = W_effT @ 1  (all PE)
  C (per chunk, MLP software-pipelined one chunk behind):
    raw = W_effT' @ x^T;  x1 = rho*(raw) + (-mu rho)*wsum + bsum + x
    LN2 stats via ones-matmuls; h2 = (x1-mu2)*rho2 (bf16)
    m = gelu(m1 @ h2); out = m2 @ m + b + x1

dtypes: everything f32r (tf32) except v_w/att/h2/mg/m1/m2 in bf16.
Simulated L2rel ~5e-3 vs fp32 reference (gate 2e-2).
LN affine transforms (g,b) are folded into weights host-side.
"""

import ml_dtypes
import numpy as np

import concourse.bass as bass
import concourse.tile as tile
from concourse import bacc, mybir
from concourse import bass_utils

F32 = mybir.dt.float32
F32R = mybir.dt.float32r
BF16 = mybir.dt.bfloat16
AF = mybir.ActivationFunctionType
ALU = mybir.AluOpType
AX = mybir.AxisListType

B, N, P = 8, 4096, 1024
P4 = P // 4          # 256
EPS = 1e-5
CH = 512             # token chunk
NCH = N // CH        # 8
KP = P // 128        # 8 channel tiles
TS = CH // 128       # 4 token sub-tiles per chunk
INV_P = 1.0 / P


def _build(loop_R: int = 1):
    nc = bacc.Bacc("TRN2", target_bir_lowering=False, debug=False)

    # ---- DRAM I/O ----
    xT_d = nc.dram_tensor("xT", [P, N], F32, kind="ExternalInput").ap()
    qk_wT_d = nc.dram_tensor("qk_wT", [P, P4], F32R, kind="ExternalInput").ap()
    qksum_d = nc.dram_tensor("qksum", [P4], F32, kind="ExternalInput").ap()
    qkb_d = nc.dram_tensor("qkb", [P4], F32, kind="ExternalInput").ap()
    v_w_d = nc.dram_tensor("v_w", [P, P], BF16, kind="ExternalInput").ap()
    v_b_d = nc.dram_tensor("v_b", [P], BF16, kind="ExternalInput").ap()
    t1_wT_d = nc.dram_tensor("t1_wT", [P4, P], F32R, kind="ExternalInput").ap()
    t1_b_d = nc.dram_tensor("t1_b", [P], F32, kind="ExternalInput").ap()
    t2_wT_d = nc.dram_tensor("t2_wT", [P4, P], F32R, kind="ExternalInput").ap()
    t2_b_d = nc.dram_tensor("t2_b", [P], F32, kind="ExternalInput").ap()
    m1_wT_d = nc.dram_tensor("m1_wT", [P, P], BF16, kind="ExternalInput").ap()
    m1_b_d = nc.dram_tensor("m1_b", [P], F32, kind="ExternalInput").ap()
    m2_wT_d = nc.dram_tensor("m2_wT", [P, P], BF16, kind="ExternalInput").ap()
    m2_b_d = nc.dram_tensor("m2_b", [P], F32, kind="ExternalInput").ap()
    outT_d = nc.dram_tensor("outT", [P, N], F32, kind="ExternalOutput").ap()

    def bcast_src(vec_ap):
        """[n] dram vector -> stride-0 partition-broadcast src AP [128, n]."""
        return bass.AP(tensor=vec_ap.tensor, offset=vec_ap.offset,
                       ap=[[0, 128], *vec_ap.ap])

    def part_bias_tiles(pool, dram_ap, name, dtype=F32):
        tiles = []
        for t in range(KP):
            bt = pool.tile([128, 1], dtype, tag=f"{name}{t}", name=f"{name}{t}")
            nc.gpsimd.dma_start(bt[:], dram_ap[t * 128:(t + 1) * 128])
            tiles.append(bt)
        return tiles

    with tile.TileContext(nc) as tc:
        with (
            tc.tile_pool(name="dram", bufs=1, space="DRAM") as dram_pool,
            tc.tile_pool(name="consts", bufs=1) as consts,
        ):
            # tiny DRAM scratch rows for layout bounces
            rho_row_d = dram_pool.tile([N], F32, name="rho_row")
            nmr_row_d = dram_pool.tile([N], F32, name="nmr_row")

            # ---- persistent constants (loaded once) ----
            ones_r = consts.tile([128, 128], F32R, tag="ones_r", name="ones_r")
            nc.vector.memset(ones_r[:], INV_P)
            ones_bf = consts.tile([128, 128], BF16, tag="ones_bf",
                                  name="ones_bf")
            nc.vector.memset(ones_bf[:], INV_P)
            ones_col = consts.tile([128, 1], F32R, tag="ones_c", name="ones_c")
            nc.vector.memset(ones_col[:], 1.0)
            ones_col = consts.tile([128, 1], F32R, tag="ones_c",
                                   name="ones_c")
            nc.vector.memset(ones_col[:], INV_P)
            eps_t = consts.tile([128, 1], F32, tag="eps", name="eps_t")
            nc.vector.memset(eps_t[:], EPS)
            m1b_t = part_bias_tiles(consts, m1_b_d, "m1b")
            m2b_t = part_bias_tiles(consts, m2_b_d, "m2b")
            t2b_t = part_bias_tiles(consts, t2_b_d, "t2b")

            from contextlib import ExitStack as _ES
            _loop_ctx = _ES()
            if loop_R > 1:
                _loop_ctx.enter_context(tc.For_i(0, loop_R, 1))

            with tc.tile_pool(name="wBC", bufs=1) as wBC:  # survives into C
                bsum_t = [wBC.tile([128, 1], F32, tag=f"bs{q}", name=f"bs{q}")
                          for q in range(KP)]
                wsum_t = [wBC.tile([128, 1], F32, tag=f"ws{q}", name=f"ws{q}")
                          for q in range(KP)]
                weff = [wBC.tile([128, P], F32R, tag=f"wf{c}", name=f"wf{c}")
                        for c in range(KP)]

                with tc.tile_pool(name="wB", bufs=1) as wB:  # closed after B
                    # ---- weights (per iteration, overlap with phase A) ----
                    qk_ext = []
                    for p in range(KP):
                        t = wB.tile([128, P4 + 2], F32R, tag=f"qke{p}",
                                    name=f"qke{p}")
                        nc.gpsimd.dma_start(t[:, :P4],
                                            qk_wT_d[p * 128:(p + 1) * 128, :])
                        nc.vector.memset(t[:, P4:P4 + 1], INV_P)
                        qk_ext.append(t)
                    qksum_bc = wB.tile([128, P4], F32, tag="qksbc",
                                       name="qksbc")
                    nc.sync.dma_start(qksum_bc[:], bcast_src(qksum_d))
                    qkb_bc = wB.tile([128, P4], F32, tag="qkbbc", name="qkbbc")
                    nc.sync.dma_start(qkb_bc[:], bcast_src(qkb_d))
                    v_w_r, t1w, t2w = [], [], []
                    for p in range(KP):
                        t = wB.tile([128, P], BF16, tag=f"vw{p}", name=f"vw{p}")
                        nc.gpsimd.dma_start(t[:],
                                            v_w_d[p * 128:(p + 1) * 128, :])
                        v_w_r.append(t)
                    for i in range(2):
                        t = wB.tile([128, P], F32R, tag=f"t1w{i}",
                                    name=f"t1w{i}")
                        nc.gpsimd.dma_start(t[:],
                                            t1_wT_d[i * 128:(i + 1) * 128, :])
                        t1w.append(t)
                        t2 = wB.tile([128, P], F32R, tag=f"t2w{i}",
                                     name=f"t2w{i}")
                        nc.gpsimd.dma_start(t2[:],
                                            t2_wT_d[i * 128:(i + 1) * 128, :])
                        t2w.append(t2)
                    t1b_bc = wB.tile([128, P], F32, tag="t1bbc", name="t1bbc")
                    nc.gpsimd.dma_start(t1b_bc[:], bcast_src(t1_b_d))
                    vb_r = part_bias_tiles(wB, v_b_d, "vb", BF16)
                    m1w, m2w = [], []
                    for p in range(KP):
                        t = wBC.tile([128, P], BF16, tag=f"m1w{p}",
                                     name=f"m1w{p}")
                        nc.gpsimd.dma_start(t[:],
                                            m1_wT_d[p * 128:(p + 1) * 128, :])
                        m1w.append(t)
                    for p in range(KP):
                        t = wBC.tile([128, P], BF16, tag=f"m2w{p}",
                                     name=f"m2w{p}")
                        nc.gpsimd.dma_start(t[:],
                                            m2_wT_d[p * 128:(p + 1) * 128, :])
                        m2w.append(t)

                    with tc.tile_pool(name="pB", bufs=1) as pB:
                        with tc.tile_pool(name="psE", bufs=1,
                                          space="PSUM") as psE:
                            e_ps = [psE.tile([128, P4], F32, tag=f"e{i}",
                                             name=f"e_ps{i}")
                                    for i in range(2)]

                            # ============ PHASE A ============
                            self_a = _phase_a(
                                nc, tc, bass, xT_d, msq_row_d, rho_row_d,
                                nmr_row_d, qk_ext, qksum_bc, qkb_bc, ones_r,
                                eps_t, e_ps)

                            # energy -> SBUF, then release PSUM banks
                            e_sb = []
                            for i in range(2):
                                t = pB.tile([128, P4], F32, tag=f"esb{i}",
                                            name=f"esb{i}")
                                nc.vector.tensor_copy(t[:], e_ps[i][:])
                                e_sb.append(t)

                        # ============ PHASE B ============
                        with tc.tile_pool(name="psB", bufs=1,
                                          space="PSUM") as psB:
                            G_T = []
                            for jc in range(2):
                                g = pB.tile([128, P], F32R, tag=f"gt{jc}",
                                            name=f"gt{jc}")
                                for kc in range(2):
                                    ks = slice(kc * 512, (kc + 1) * 512)
                                    ps = psB.tile([128, 512], F32, tag="a1",
                                                  name="a1_ps", bufs=2)
                                    for qi in range(2):
                                        nc.tensor.matmul(
                                            ps[:],
                                            e_sb[qi][:].bitcast(F32R)[
                                                :, jc * 128:(jc + 1) * 128],
                                            t1w[qi][:, ks],
                                            start=(qi == 0), stop=(qi == 1),
                                        )
                                    tmp = pB.tile([128, 512], F32, tag="a1t",
                                                  name="a1t", bufs=2)
                                    nc.vector.tensor_tensor(
                                        tmp[:], ps[:], t1b_bc[:, ks], ALU.add)
                                    nc.scalar.activation(g[:, ks], tmp[:],
                                                         AF.Gelu)
                                G_T.append(g)
                            att_r = []
                            for o in range(KP):
                                att2 = pB.tile([128, P], F32, tag="att2",
                                               name="att2", bufs=2)
                                for kc in range(2):
                                    ks = slice(kc * 512, (kc + 1) * 512)
                                    ps = psB.tile([128, 512], F32, tag="a2",
                                                  name="a2_ps", bufs=2)
                                    for ji in range(2):
                                        nc.tensor.matmul(
                                            ps[:],
                                            t2w[ji][:, o * 128:(o + 1) * 128],
                                            G_T[ji][:, ks],
                                            start=(ji == 0), stop=(ji == 1),
                                        )
                                    nc.scalar.activation(
                                        att2[:, ks], ps[:], AF.Identity,
                                        bias=t2b_t[o][:])
                                negmax = pB.tile([128, 1], F32, tag="negmax",
                                                 name="negmax", bufs=2)
                                nc.vector.tensor_reduce(
                                    negmax[:], att2[:], axis=AX.X, op=ALU.max,
                                    negate=True)
                                esum = pB.tile([128, 1], F32, tag="esum",
                                               name="esum", bufs=2)
                                expv = pB.tile([128, P], F32, tag="expv",
                                               name="expv", bufs=2)
                                nc.scalar.activation(
                                    expv[:], att2[:], AF.Exp, bias=negmax[:],
                                    accum_out=esum[:])
                                rec = pB.tile([128, 1], F32, tag="rec",
                                              name="rec", bufs=2)
                                nc.vector.reciprocal(rec[:], esum[:])
                                at = pB.tile([128, P], BF16, tag=f"attr{o}",
                                             name=f"attr{o}")
                                nc.vector.tensor_scalar_mul(at[:], expv[:],
                                                            rec[:])
                                att_r.append(at)
                            # bsum[q] = att^T v_b
                            for qt in range(KP):
                                ps = psB.tile([128, 1], F32, tag="bsp",
                                              name="bsp", bufs=1)
                                for pt in range(KP):
                                    nc.tensor.matmul(
                                        ps[:],
                                        att_r[pt][:, qt * 128:(qt + 1) * 128],
                                        vb_r[pt][:], start=(pt == 0),
                                        stop=(pt == KP - 1),
                                    )
                                nc.vector.tensor_copy(bsum_t[qt][:], ps[:])
                            # W_effT[c, q] = sum_p v_w[p, c] att[p, q]
                            for qh in range(2):
                                qs = slice(qh * 512, (qh + 1) * 512)
                                for ct in range(KP):
                                    ps = psB.tile([128, 512], F32, tag="wfp",
                                                  name="wfp", bufs=2)
                                    for pt in range(KP):
                                        nc.tensor.matmul(
                                            ps[:],
                                            v_w_r[pt][
                                                :, ct * 128:(ct + 1) * 128],
                                            att_r[pt][:, qs],
                                            start=(pt == 0),
                                            stop=(pt == KP - 1),
                                        )
                                    nc.vector.tensor_copy(weff[ct][:, qs],
                                                          ps[:])
                            # wsum[q] = sum_c W_effT[c, q]
                            for qt in range(KP):
                                ps = psB.tile([128, 1], F32, tag="wsp",
                                              name="wsp", bufs=1)
                                for ct in range(KP):
                                    nc.tensor.matmul(
                                        ps[:],
                                        weff[ct][:, qt * 128:(qt + 1) * 128],
                                        ones_col[:], start=(ct == 0),
                                        stop=(ct == KP - 1),
                                    )
                                nc.vector.tensor_copy(wsum_t[qt][:], ps[:])

                # ============ PHASE C ============
                with (
                    tc.tile_pool(name="pC", bufs=1) as pC,
                    tc.tile_pool(name="psC", bufs=1, space="PSUM") as psC,
                ):
                    prev = None

                    def emit_mlp(h2p, x1p, csp):
                        mg = []
                        for j in range(KP):
                            ps = psC.tile([128, CH], F32, tag="m1",
                                          name="m1_ps", bufs=2)
                            for ct in range(KP):
                                nc.tensor.matmul(
                                    ps[:], m1w[ct][:, j * 128:(j + 1) * 128],
                                    h2p[ct][:], start=(ct == 0),
                                    stop=(ct == KP - 1),
                                )
                            g = pC.tile([128, CH], BF16, tag=f"mg{j}",
                                        name=f"mg{j}")
                            nc.scalar.activation(g[:], ps[:], AF.Gelu,
                                                 bias=m1b_t[j][:])
                            mg.append(g)
                        for o in range(KP):
                            ps = psC.tile([128, CH], F32, tag="m2",
                                          name="m2_ps", bufs=2)
                            for j in range(KP):
                                nc.tensor.matmul(
                                    ps[:], m2w[j][:, o * 128:(o + 1) * 128],
                                    mg[j][:], start=(j == 0),
                                    stop=(j == KP - 1),
                                )
                            mo = pC.tile([128, CH], F32, tag="mo", name="mo",
                                         bufs=3)
                            nc.vector.scalar_tensor_tensor(
                                mo[:], ps[:], m2b_t[o][:], x1p[o][:],
                                op0=ALU.add, op1=ALU.add,
                            )
                            nc.sync.dma_start(
                                outT_d[o * 128:(o + 1) * 128, csp], mo[:])

                    for c in range(NCH):
                        cs = slice(c * CH, (c + 1) * CH)
                        xt = []
                        for p in range(KP):
                            t = pC.tile([128, CH], F32, tag=f"cxt{p}",
                                        name=f"cxt{p}", bufs=2)
                            nc.sync.dma_start(t[:],
                                              xT_d[p * 128:(p + 1) * 128, cs])
                            xt.append(t)
                        rb = pC.tile([128, CH], F32, tag="rb", name="rb",
                                     bufs=2)
                        r = rho_row_d[c * CH:(c + 1) * CH]
                        nc.scalar.dma_start(
                            rb[:], bass.AP(tensor=r.tensor, offset=r.offset,
                                           ap=[[0, 128], [1, CH]]))
                        nmb = pC.tile([128, CH], F32, tag="nmb", name="nmb",
                                      bufs=2)
                        r = nmr_row_d[c * CH:(c + 1) * CH]
                        nc.scalar.dma_start(
                            nmb[:], bass.AP(tensor=r.tensor, offset=r.offset,
                                            ap=[[0, 128], [1, CH]]))
                        x1, sq2 = [], []
                        for o in range(KP):
                            ps = psC.tile([128, CH], F32, tag="tout",
                                          name="tout_ps", bufs=2)
                            for ct in range(KP):
                                nc.tensor.matmul(
                                    ps[:], weff[ct][:, o * 128:(o + 1) * 128],
                                    xt[ct][:].bitcast(F32R),
                                    start=(ct == 0), stop=(ct == KP - 1),
                                )
                            v1 = pC.tile([128, CH], F32, tag="v1", name="v1",
                                         bufs=2)
                            nc.vector.tensor_mul(v1[:], ps[:], rb[:])
                            v2 = pC.tile([128, CH], F32, tag="v2", name="v2",
                                         bufs=2)
                            nc.vector.scalar_tensor_tensor(
                                v2[:], nmb[:], wsum_t[o][:], v1[:],
                                op0=ALU.mult, op1=ALU.add,
                            )
                            xo = pC.tile([128, CH], F32, tag=f"x1_{o}",
                                         name=f"x1_{o}", bufs=2)
                            nc.vector.scalar_tensor_tensor(
                                xo[:], v2[:], bsum_t[o][:], xt[o][:],
                                op0=ALU.add, op1=ALU.add,
                            )
                            x1.append(xo)
                            sq = pC.tile([128, CH], BF16, tag=f"sq2_{o}",
                                         name=f"sq2_{o}")
                            nc.scalar.activation(sq[:], xo[:], AF.Square)
                            sq2.append(sq)
                        sum2 = psC.tile([128, CH], F32, tag="s2", name="s2")
                        for o in range(KP):
                            nc.tensor.matmul(sum2[:], ones_r[:],
                                             x1[o][:].bitcast(F32R),
                                             start=(o == 0),
                                             stop=(o == KP - 1))
                        msq2 = psC.tile([128, CH], F32, tag="q2", name="q2")
                        for o in range(KP):
                            nc.tensor.matmul(msq2[:], ones_bf[:], sq2[o][:],
                                             start=(o == 0),
                                             stop=(o == KP - 1))
                        mu2 = pC.tile([128, CH], F32, tag="mu2", name="mu2",
                                      bufs=2)
                        nc.vector.tensor_copy(mu2[:], sum2[:])
                        tv = pC.tile([128, CH], F32, tag="tv", name="tv",
                                     bufs=2)
                        nc.vector.tensor_mul(tv[:], mu2[:], mu2[:])
                        vr = pC.tile([128, CH], F32, tag="vr", name="vr",
                                     bufs=2)
                        nc.vector.tensor_tensor(vr[:], msq2[:], tv[:],
                                                ALU.subtract)
                        sd = pC.tile([128, CH], F32, tag="sd", name="sd",
                                     bufs=2)
                        nc.scalar.activation(sd[:], vr[:], AF.Sqrt,
                                             bias=eps_t[:])
                        rho2 = pC.tile([128, CH], F32, tag="rho2", name="rho2",
                                       bufs=2)
                        nc.vector.reciprocal(rho2[:], sd[:])
                        h2 = []
                        for ct in range(KP):
                            ht = pC.tile([128, CH], F32, tag="h2t", name="h2t",
                                         bufs=2)
                            nc.vector.tensor_tensor(ht[:], x1[ct][:], mu2[:],
                                                    ALU.subtract)
                            h = pC.tile([128, CH], BF16, tag=f"h2_{ct}",
                                        name=f"h2_{ct}", bufs=2)
                            nc.vector.tensor_mul(h[:], ht[:], rho2[:])
                            h2.append(h)
                        if prev is not None:
                            emit_mlp(*prev)
                        prev = (h2, x1, cs)
                    emit_mlp(*prev)

            _loop_ctx.close()

    nc.compile()
    return nc


def _phase_a(nc, tc, bass, xT_d, rho_row_d, nmr_row_d, qk_ext,
             qksum_bc, qkb_bc, ones_r, eps_t, e_ps):
    with (
        tc.tile_pool(name="pA", bufs=1) as pA,
        tc.tile_pool(name="psA", bufs=1, space="PSUM") as psA,
    ):
        rho_t = pA.tile([128, TS * NCH], F32, tag="rho_t", name="rho_t")
        nmr_t = pA.tile([128, TS * NCH], F32, tag="nmr_t", name="nmr_t")
        mu_t = pA.tile([128, TS * NCH], F32, tag="mu_t", name="mu_t")
        prev_xqc = None

        def emit_energy(xqc, cc):
            for ts in range(TS):
                for qh in range(2):
                    nc.tensor.matmul(
                        e_ps[qh][:],
                        xqc[ts][:, qh * 128:(qh + 1) * 128],
                        xqc[ts][:],
                        start=(cc == 0 and ts == 0),
                        stop=(cc == NCH - 1 and ts == TS - 1),
                        skip_group_check=True,
                    )

        for c in range(NCH):
            cs = slice(c * CH, (c + 1) * CH)
            xt = []
            for p in range(KP):
                t = pA.tile([128, CH], F32, tag=f"xt{p}", name=f"xt{p}",
                            bufs=2)
                nc.sync.dma_start(t[:], xT_d[p * 128:(p + 1) * 128, cs])
                xt.append(t)
            # msq directly in token-part layout: tiny column matmuls
            # (stationary = x^2 sub-tile, moving = ones/P column)
            msq_ps4 = [psA.tile([128, 2], F32, tag=f"msqp{ts}",
                                name=f"msqp{ts}", bufs=1)
                       for ts in range(TS)]
            for p in range(KP):
                sq = pA.tile([128, CH], F32R, tag=f"sq{p % 2}",
                             name=f"sq{p % 2}", bufs=2)
                nc.scalar.activation(sq[:], xt[p][:], AF.Square)
                for ts in range(TS):
                    nc.tensor.matmul(
                        msq_ps4[ts][:],
                        sq[:, ts * 128:(ts + 1) * 128],
                        ones_col[:],
                        start=(p == 0), stop=(p == KP - 1),
                        skip_group_check=True,
                    )
            # x_q (token-part) + mean ride-along; per-ts rho + correction
            xq_c = []
            for ts in range(TS):
                xq_ps = psA.tile([128, P4 + 2], F32, tag="xq", name="xq_ps",
                                 bufs=2)
                for p in range(KP):
                    nc.tensor.matmul(
                        xq_ps[:],
                        xt[p][:].bitcast(F32R)[:, ts * 128:(ts + 1) * 128],
                        qk_ext[p][:],
                        start=(p == 0), stop=(p == KP - 1),
                    )
                col = c * TS + ts
                mu_c = mu_t[:, col:col + 1]
                nc.vector.tensor_copy(mu_c, xq_ps[:, P4:P4 + 1])
                tmp1 = pA.tile([128, 1], F32, tag="tmp1", name="tmp1", bufs=2)
                nc.vector.tensor_mul(tmp1[:], mu_c, mu_c)
                var1 = pA.tile([128, 1], F32, tag="var1", name="var1", bufs=2)
                nc.vector.tensor_tensor(var1[:], msq_ps4[ts][:, 0:1],
                                        tmp1[:], ALU.subtract)
                sd1 = pA.tile([128, 1], F32, tag="sd1", name="sd1", bufs=2)
                nc.scalar.activation(sd1[:], var1[:], AF.Sqrt, bias=eps_t[:])
                rho_c = rho_t[:, col:col + 1]
                nc.vector.reciprocal(rho_c, sd1[:])
                nmr_c = nmr_t[:, col:col + 1]
                nc.vector.scalar_tensor_tensor(
                    nmr_c, mu_c, -1.0, rho_c, op0=ALU.mult, op1=ALU.mult)
                qtmp = pA.tile([128, P4], F32, tag="qtmp", name="qtmp",
                               bufs=2)
                nc.vector.scalar_tensor_tensor(
                    qtmp[:], qksum_bc[:], nmr_c, qkb_bc[:],
                    op0=ALU.mult, op1=ALU.add,
                )
                xc = pA.tile([128, P4], F32R, tag=f"xqc{ts}", name=f"xqc{ts}",
                             bufs=2)
                nc.vector.scalar_tensor_tensor(
                    xc[:], xq_ps[:, :P4], rho_c, qtmp[:],
                    op0=ALU.mult, op1=ALU.add,
                )
                xq_c.append(xc)
            if prev_xqc is not None:
                emit_energy(prev_xqc, c - 1)
            prev_xqc = xq_c
        emit_energy(prev_xqc, NCH - 1)
        # per-token rows -> DRAM (token-part -> row layout)
        for src, dst in ((rho_t, rho_row_d), (nmr_t, nmr_row_d)):
            d = dst[:]
            nc.sync.dma_start(
                bass.AP(tensor=d.tensor, offset=d.offset,
                        ap=[[1, 128], [128, TS * NCH]]),
                src[:],
            )


_CACHE = {}


def _get_nc(loop_R=1):
    if loop_R not in _CACHE:
        _CACHE[loop_R] = _build(loop_R)
    return _CACHE[loop_R]


def _prep_base(inputs):
    f32 = lambda k: np.asarray(inputs[k], np.float32)
    bf16 = ml_dtypes.bfloat16
    g1, b1 = f32("ln1_g"), f32("ln1_b")
    g2, b2 = f32("ln2_g"), f32("ln2_b")
    qk_w = f32("qk_w") * g1[None, :]
    v_w = f32("v_w") * g1[None, :]
    m1_w = f32("m1_w") * g2[None, :]
    return {
        "qk_wT": np.ascontiguousarray(qk_w.T),
        "qksum": np.ascontiguousarray(qk_w.sum(axis=1)),
        "qkb": np.ascontiguousarray(f32("qk_w") @ b1),
        "v_w": np.ascontiguousarray(v_w).astype(bf16),
        "v_b": np.ascontiguousarray(
            f32("v_b") + f32("v_w") @ b1).astype(bf16),
        "t1_wT": np.ascontiguousarray(f32("t1_w").T),
        "t1_b": np.ascontiguousarray(f32("t1_b")),
        "t2_wT": np.ascontiguousarray(f32("t2_w").T),
        "t2_b": np.ascontiguousarray(f32("t2_b")),
        "m1_wT": np.ascontiguousarray(m1_w.T).astype(bf16),
        "m1_b": np.ascontiguousarray(f32("m1_b") + f32("m1_w") @ b2),
        "m2_wT": np.ascontiguousarray(f32("m2_w").T).astype(bf16),
        "m2_b": np.ascontiguousarray(f32("m2_b")),
    }


def kernel(**inputs):
    return _kernel_impl(inputs, loop_R=1)


def _kernel_impl(inputs, loop_R=1):
    x = np.ascontiguousarray(np.asarray(inputs["x"], np.float32))
    assert x.shape == (B, N, P), x.shape
    nc = _get_nc(loop_R)
    base = _prep_base(inputs)
    in_maps = []
    for b in range(B):
        m = dict(base)
        m["xT"] = np.ascontiguousarray(x[b].T)
        in_maps.append(m)
    res = bass_utils.run_bass_kernel_spmd(nc, in_maps, core_ids=list(range(B)))
    out = np.empty((B, N, P), np.float32)
    for b in range(B):
        out[b] = res.results[b]["outT"].T
    return out


if __name__ == "__main__":
    import sys
    import time

    sys.path.insert(0, "/root/problem")
    import reference as refmod

    inputs = {k: np.asarray(v) for k, v in refmod.setup_inputs().items()}
    t0 = time.time()
    got = kernel(**inputs)
    print(f"kernel() took {time.time() - t0:.1f}s (incl compile)")
    exp = np.asarray(refmod.reference(**inputs))
    err = np.abs(got - exp)
    l2 = np.linalg.norm(got - exp) / np.linalg.norm(exp)
    print(f"absmax={err.max():.3e} L2rel={l2:.3e}")


# revision 4
# speedup vs baseline: 4.3000x; 1.3539x over previous
"""Trainium2 Bass kernel for nn_Enhanced_transformer (dense transformer block).

Strategy (v2)
-------------
Data-parallel: one batch element per NeuronCore (8 cores), no collectives.
Channel-major layout [channel-part, token-free] throughout. All GEMMs run at
1 cyc/row (f32r or bf16); LN is fused as post-GEMM affine corrections so the
normalized tensor h is never materialized; the value path is algebraically
restructured:

    t_out^T = att^T @ x_v^T = (att^T v_w) @ h^T = W_eff @ h^T

which replaces the two big GEMMs (x_v then t_out, 2 x 262k cyc) with one
small build GEMM (65k) + one big apply GEMM (262k), and eliminates the x_v
[P,N] spill entirely. No DRAM activation spills at all: only x is (re)read
in phases A and C; per-token LN1 stats bounce through tiny DRAM rows to
switch between token-partition and broadcast-row layouts.

  A (per 512-token chunk):  sq=x^2 (ACT), msq = ones@sq (PE, broadcast),
    x_q = x^T' @ qk_ext (PE, token-part, mean rides as an extra column),
    rho/(-mu rho) in token layout (DVE), x_q corrected (DVE),
    energy += x_q' x_q (PE, PSUM accum across chunks, 1-chunk pipelined)
  B: a1^T = E @ t1_wT (+bias, gelu) -> att2 = t2_wT' @ G (+bias) -> softmax
     -> att (bf16);  bsum = att^T v_b, W_effT = v_w^T-stat @ att,
     wsum = W_effT @ 1  (all PE)
  C (per chunk, MLP software-pipelined one chunk behind):
    raw = W_effT' @ x^T;  x1 = rho*(raw) + (-mu rho)*wsum + bsum + x
    LN2 stats via ones-matmuls; h2 = (x1-mu2)*rho2 (bf16)
    m = gelu(m1 @ h2); out = m2 @ m + b + x1

dtypes: everything f32r (tf32) except v_w/att/h2/mg/m1/m2 in bf16.
Simulated L2rel ~5e-3 vs fp32 reference (gate 2e-2).
LN affine transforms (g,b) are folded into weights host-side.
"""

import ml_dtypes
import numpy as np

import concourse.bass as bass
import concourse.tile as tile
from concourse import bacc, mybir
from concourse import bass_utils

F32 = mybir.dt.float32
F32R = mybir.dt.float32r
BF16 = mybir.dt.bfloat16
AF = mybir.ActivationFunctionType
ALU = mybir.AluOpType
AX = mybir.AxisListType

B, N, P = 8, 4096, 1024
P4 = P // 4          # 256
EPS = 1e-5
CH = 512             # token chunk
NCH = N // CH        # 8
KP = P // 128        # 8 channel tiles
TS = CH // 128       # 4 token sub-tiles per chunk
INV_P = 1.0 / P


def _build(loop_R: int = 1):
    nc = bacc.Bacc("TRN2", target_bir_lowering=False, debug=False)

    # ---- DRAM I/O ----
    xT_d = nc.dram_tensor("xT", [P, N], F32, kind="ExternalInput").ap()
    qk_wT_d = nc.dram_tensor("qk_wT", [P, P4], F32R, kind="ExternalInput").ap()
    qksum_d = nc.dram_tensor("qksum", [P4], F32, kind="ExternalInput").ap()
    qkb_d = nc.dram_tensor("qkb", [P4], F32, kind="ExternalInput").ap()
    v_w_d = nc.dram_tensor("v_w", [P, P], BF16, kind="ExternalInput").ap()
    v_b_d = nc.dram_tensor("v_b", [P], BF16, kind="ExternalInput").ap()
    t1_wT_d = nc.dram_tensor("t1_wT", [P4, P], F32R, kind="ExternalInput").ap()
    t1_b_d = nc.dram_tensor("t1_b", [P], F32, kind="ExternalInput").ap()
    t2_wT_d = nc.dram_tensor("t2_wT", [P4, P], F32R, kind="ExternalInput").ap()
    m1_wT_d = nc.dram_tensor("m1_wT", [P, P], BF16, kind="ExternalInput").ap()
    m1_b_d = nc.dram_tensor("m1_b", [P], F32, kind="ExternalInput").ap()
    m2_wT_d = nc.dram_tensor("m2_wT", [P, P], BF16, kind="ExternalInput").ap()
    m2_b_d = nc.dram_tensor("m2_b", [P], F32, kind="ExternalInput").ap()
    outT_d = nc.dram_tensor("outT", [P, N], F32, kind="ExternalOutput").ap()

    def bcast_src(vec_ap):
        """[n] dram vector -> stride-0 partition-broadcast src AP [128, n]."""
        return bass.AP(tensor=vec_ap.tensor, offset=vec_ap.offset,
                       ap=[[0, 128], *vec_ap.ap])

    def part_bias_tiles(pool, dram_ap, name, dtype=F32):
        tiles = []
        for t in range(KP):
            bt = pool.tile([128, 1], dtype, tag=f"{name}{t}", name=f"{name}{t}")
            nc.gpsimd.dma_start(bt[:], dram_ap[t * 128:(t + 1) * 128])
            tiles.append(bt)
        return tiles

    with tile.TileContext(nc) as tc:
        with (
            tc.tile_pool(name="dram", bufs=1, space="DRAM") as dram_pool,
            tc.tile_pool(name="consts", bufs=1) as consts,
        ):
            # tiny DRAM scratch rows for layout bounces
            rho_row_d = dram_pool.tile([N], F32, name="rho_row")
            nmr_row_d = dram_pool.tile([N], F32, name="nmr_row")

            # ---- persistent constants (loaded once) ----
            ones_r = consts.tile([128, 128], F32R, tag="ones_r", name="ones_r")
            nc.vector.memset(ones_r[:], INV_P)
            ones_bf = consts.tile([128, 128], BF16, tag="ones_bf",
                                  name="ones_bf")
            nc.vector.memset(ones_bf[:], INV_P)
            ones_col = consts.tile([128, 1], F32R, tag="ones_c", name="ones_c")
            nc.vector.memset(ones_col[:], 1.0)
            ones_col = consts.tile([128, 1], F32R, tag="ones_c",
                                   name="ones_c")
            nc.vector.memset(ones_col[:], INV_P)
            eps_t = consts.tile([128, 1], F32, tag="eps", name="eps_t")
            nc.vector.memset(eps_t[:], EPS)
            m1b_t = part_bias_tiles(consts, m1_b_d, "m1b")
            m2b_t = part_bias_tiles(consts, m2_b_d, "m2b")

            from contextlib import ExitStack as _ES
            _loop_ctx = _ES()
            if loop_R > 1:
                _loop_ctx.enter_context(tc.For_i(0, loop_R, 1))

            with tc.tile_pool(name="wBC", bufs=1) as wBC:  # survives into C
                bsum_t = [wBC.tile([128, 1], F32, tag=f"bs{q}", name=f"bs{q}")
                          for q in range(KP)]
                wsum_t = [wBC.tile([128, 1], F32, tag=f"ws{q}", name=f"ws{q}")
                          for q in range(KP)]
                weff = [wBC.tile([128, P], F32R, tag=f"wf{c}", name=f"wf{c}")
                        for c in range(KP)]

                with tc.tile_pool(name="wB", bufs=1) as wB:  # closed after B
                    # ---- weights (per iteration, overlap with phase A) ----
                    qk_ext = []
                    for p in range(KP):
                        t = wB.tile([128, P4 + 2], F32R, tag=f"qke{p}",
                                    name=f"qke{p}")
                        nc.gpsimd.dma_start(t[:, :P4],
                                            qk_wT_d[p * 128:(p + 1) * 128, :])
                        nc.vector.memset(t[:, P4:P4 + 1], INV_P)
                        qk_ext.append(t)
                    qksum_bc = wB.tile([128, P4], F32, tag="qksbc",
                                       name="qksbc")
                    nc.sync.dma_start(qksum_bc[:], bcast_src(qksum_d))
                    qkb_bc = wB.tile([128, P4], F32, tag="qkbbc", name="qkbbc")
                    nc.sync.dma_start(qkb_bc[:], bcast_src(qkb_d))
                    v_w_r, t1w, t2w = [], [], []
                    for p in range(KP):
                        t = wB.tile([128, P], BF16, tag=f"vw{p}", name=f"vw{p}")
                        nc.gpsimd.dma_start(t[:],
                                            v_w_d[p * 128:(p + 1) * 128, :])
                        v_w_r.append(t)
                    for i in range(2):
                        t = wB.tile([128, P], F32R, tag=f"t1w{i}",
                                    name=f"t1w{i}")
                        nc.gpsimd.dma_start(t[:],
                                            t1_wT_d[i * 128:(i + 1) * 128, :])
                        t1w.append(t)
                        t2 = wB.tile([128, P], F32R, tag=f"t2w{i}",
                                     name=f"t2w{i}")
                        nc.gpsimd.dma_start(t2[:],
                                            t2_wT_d[i * 128:(i + 1) * 128, :])
                        t2w.append(t2)
                    t1b_bc = wB.tile([128, P], F32, tag="t1bbc", name="t1bbc")
                    nc.gpsimd.dma_start(t1b_bc[:], bcast_src(t1_b_d))
                    vb_r = part_bias_tiles(wB, v_b_d, "vb", BF16)
                    m1w, m2w = [], []
                    for p in range(KP):
                        t = wBC.tile([128, P], BF16, tag=f"m1w{p}",
                                     name=f"m1w{p}")
                        nc.gpsimd.dma_start(t[:],
                                            m1_wT_d[p * 128:(p + 1) * 128, :])
                        m1w.append(t)
                    for p in range(KP):
                        t = wBC.tile([128, P], BF16, tag=f"m2w{p}",
                                     name=f"m2w{p}")
                        nc.gpsimd.dma_start(t[:],
                                            m2_wT_d[p * 128:(p + 1) * 128, :])
                        m2w.append(t)

                    with tc.tile_pool(name="pB", bufs=1) as pB:
                        with tc.tile_pool(name="psE", bufs=1,
                                          space="PSUM") as psE:
                            e_ps = [psE.tile([128, P4], F32, tag=f"e{i}",
                                             name=f"e_ps{i}")
                                    for i in range(2)]

                            # ============ PHASE A ============
                            self_a = _phase_a(
                                nc, tc, bass, xT_d, msq_row_d, rho_row_d,
                                nmr_row_d, qk_ext, qksum_bc, qkb_bc, ones_r,
                                eps_t, e_ps)

                            # energy -> SBUF, then release PSUM banks
                            e_sb = []
                            for i in range(2):
                                t = pB.tile([128, P4], F32, tag=f"esb{i}",
                                            name=f"esb{i}")
                                nc.vector.tensor_copy(t[:], e_ps[i][:])
                                e_sb.append(t)

                        # ============ PHASE B ============
                        with tc.tile_pool(name="psB", bufs=1,
                                          space="PSUM") as psB:
                            G_T = []
                            for jc in range(2):
                                g = pB.tile([128, P], F32R, tag=f"gt{jc}",
                                            name=f"gt{jc}")
                                for kc in range(2):
                                    ks = slice(kc * 512, (kc + 1) * 512)
                                    ps = psB.tile([128, 512], F32, tag="a1",
                                                  name="a1_ps", bufs=2)
                                    for qi in range(2):
                                        nc.tensor.matmul(
                                            ps[:],
                                            e_sb[qi][:].bitcast(F32R)[
                                                :, jc * 128:(jc + 1) * 128],
                                            t1w[qi][:, ks],
                                            start=(qi == 0), stop=(qi == 1),
                                        )
                                    tmp = pB.tile([128, 512], F32, tag="a1t",
                                                  name="a1t", bufs=2)
                                    nc.vector.tensor_tensor(
                                        tmp[:], ps[:], t1b_bc[:, ks], ALU.add)
                                    nc.scalar.activation(g[:, ks], tmp[:],
                                                         AF.Gelu)
                                G_T.append(g)
                            # t2_b is constant along the softmax axis ->
                            # softmax is invariant to it; skip the bias and
                            # run max/exp straight off the PSUM halves.
                            att_r = []
                            for o in range(KP):
                                psh = []
                                nmh = []
                                for kc in range(2):
                                    ks = slice(kc * 512, (kc + 1) * 512)
                                    ps = psB.tile([128, 512], F32, tag="a2",
                                                  name="a2_ps", bufs=4)
                                    for ji in range(2):
                                        nc.tensor.matmul(
                                            ps[:],
                                            t2w[ji][:, o * 128:(o + 1) * 128],
                                            G_T[ji][:, ks],
                                            start=(ji == 0), stop=(ji == 1),
                                        )
                                    nm = pB.tile([128, 1], F32,
                                                 tag=f"nmh{kc}",
                                                 name=f"nmh{kc}", bufs=2)
                                    nc.vector.tensor_reduce(
                                        nm[:], ps[:], axis=AX.X, op=ALU.max,
                                        negate=True)
                                    psh.append(ps)
                                    nmh.append(nm)
                                negmax = pB.tile([128, 1], F32, tag="negmax",
                                                 name="negmax", bufs=2)
                                nc.vector.tensor_tensor(negmax[:], nmh[0][:],
                                                        nmh[1][:], ALU.min)
                                expv = pB.tile([128, P], F32, tag="expv",
                                               name="expv", bufs=2)
                                esh = []
                                for kc in range(2):
                                    ks = slice(kc * 512, (kc + 1) * 512)
                                    es = pB.tile([128, 1], F32,
                                                 tag=f"esh{kc}",
                                                 name=f"esh{kc}", bufs=2)
                                    nc.scalar.activation(
                                        expv[:, ks], psh[kc][:], AF.Exp,
                                        bias=negmax[:], accum_out=es[:])
                                    esh.append(es)
                                esum = pB.tile([128, 1], F32, tag="esum",
                                               name="esum", bufs=2)
                                nc.vector.tensor_tensor(esum[:], esh[0][:],
                                                        esh[1][:], ALU.add)
                                rec = pB.tile([128, 1], F32, tag="rec",
                                              name="rec", bufs=2)
                                nc.vector.reciprocal(rec[:], esum[:])
                                at = pB.tile([128, P], BF16, tag=f"attr{o}",
                                             name=f"attr{o}")
                                nc.vector.tensor_scalar_mul(at[:], expv[:],
                                                            rec[:])
                                att_r.append(at)
                            # bsum[q] = att^T v_b
                            for qt in range(KP):
                                ps = psB.tile([128, 1], F32, tag="bsp",
                                              name="bsp", bufs=1)
                                for pt in range(KP):
                                    nc.tensor.matmul(
                                        ps[:],
                                        att_r[pt][:, qt * 128:(qt + 1) * 128],
                                        vb_r[pt][:], start=(pt == 0),
                                        stop=(pt == KP - 1),
                                    )
                                nc.vector.tensor_copy(bsum_t[qt][:], ps[:])
                            # W_effT[c, q] = sum_p v_w[p, c] att[p, q]
                            for qh in range(2):
                                qs = slice(qh * 512, (qh + 1) * 512)
                                for ct in range(KP):
                                    ps = psB.tile([128, 512], F32, tag="wfp",
                                                  name="wfp", bufs=2)
                                    for pt in range(KP):
                                        nc.tensor.matmul(
                                            ps[:],
                                            v_w_r[pt][
                                                :, ct * 128:(ct + 1) * 128],
                                            att_r[pt][:, qs],
                                            start=(pt == 0),
                                            stop=(pt == KP - 1),
                                        )
                                    nc.vector.tensor_copy(weff[ct][:, qs],
                                                          ps[:])
                            # wsum[q] = sum_c W_effT[c, q]
                            for qt in range(KP):
                                ps = psB.tile([128, 1], F32, tag="wsp",
                                              name="wsp", bufs=1)
                                for ct in range(KP):
                                    nc.tensor.matmul(
                                        ps[:],
                                        weff[ct][:, qt * 128:(qt + 1) * 128],
                                        ones_col[:], start=(ct == 0),
                                        stop=(ct == KP - 1),
                                    )
                                nc.vector.tensor_copy(wsum_t[qt][:], ps[:])

                # ============ PHASE C ============
                with (
                    tc.tile_pool(name="pC", bufs=1) as pC,
                    tc.tile_pool(name="psC", bufs=1, space="PSUM") as psC,
                ):
                    prev = None

                    def emit_mlp(h2p, x1p, csp):
                        mg = []
                        for j in range(KP):
                            ps = psC.tile([128, CH], F32, tag="m1",
                                          name="m1_ps", bufs=2)
                            for ct in range(KP):
                                nc.tensor.matmul(
                                    ps[:], m1w[ct][:, j * 128:(j + 1) * 128],
                                    h2p[ct][:], start=(ct == 0),
                                    stop=(ct == KP - 1),
                                )
                            g = pC.tile([128, CH], BF16, tag=f"mg{j}",
                                        name=f"mg{j}")
                            nc.scalar.activation(g[:], ps[:], AF.Gelu,
                                                 bias=m1b_t[j][:])
                            mg.append(g)
                        for o in range(KP):
                            ps = psC.tile([128, CH], F32, tag="m2",
                                          name="m2_ps", bufs=2)
                            for j in range(KP):
                                nc.tensor.matmul(
                                    ps[:], m2w[j][:, o * 128:(o + 1) * 128],
                                    mg[j][:], start=(j == 0),
                                    stop=(j == KP - 1),
                                )
                            mo = pC.tile([128, CH], F32, tag="mo", name="mo",
                                         bufs=3)
                            nc.vector.scalar_tensor_tensor(
                                mo[:], ps[:], m2b_t[o][:], x1p[o][:],
                                op0=ALU.add, op1=ALU.add,
                            )
                            nc.sync.dma_start(
                                outT_d[o * 128:(o + 1) * 128, csp], mo[:])

                    for c in range(NCH):
                        cs = slice(c * CH, (c + 1) * CH)
                        xt = []
                        for p in range(KP):
                            t = pC.tile([128, CH], F32, tag=f"cxt{p}",
                                        name=f"cxt{p}", bufs=2)
                            nc.sync.dma_start(t[:],
                                              xT_d[p * 128:(p + 1) * 128, cs])
                            xt.append(t)
                        rb = pC.tile([128, CH], F32, tag="rb", name="rb",
                                     bufs=2)
                        r = rho_row_d[c * CH:(c + 1) * CH]
                        nc.scalar.dma_start(
                            rb[:], bass.AP(tensor=r.tensor, offset=r.offset,
                                           ap=[[0, 128], [1, CH]]))
                        nmb = pC.tile([128, CH], F32, tag="nmb", name="nmb",
                                      bufs=2)
                        r = nmr_row_d[c * CH:(c + 1) * CH]
                        nc.scalar.dma_start(
                            nmb[:], bass.AP(tensor=r.tensor, offset=r.offset,
                                            ap=[[0, 128], [1, CH]]))
                        x1, sq2 = [], []
                        for o in range(KP):
                            ps = psC.tile([128, CH], F32, tag="tout",
                                          name="tout_ps", bufs=2)
                            for ct in range(KP):
                                nc.tensor.matmul(
                                    ps[:], weff[ct][:, o * 128:(o + 1) * 128],
                                    xt[ct][:].bitcast(F32R),
                                    start=(ct == 0), stop=(ct == KP - 1),
                                )
                            v1 = pC.tile([128, CH], F32, tag="v1", name="v1",
                                         bufs=2)
                            nc.vector.tensor_mul(v1[:], ps[:], rb[:])
                            v2 = pC.tile([128, CH], F32, tag="v2", name="v2",
                                         bufs=2)
                            nc.vector.scalar_tensor_tensor(
                                v2[:], nmb[:], wsum_t[o][:], v1[:],
                                op0=ALU.mult, op1=ALU.add,
                            )
                            xo = pC.tile([128, CH], F32, tag=f"x1_{o}",
                                         name=f"x1_{o}", bufs=2)
                            nc.vector.scalar_tensor_tensor(
                                xo[:], v2[:], bsum_t[o][:], xt[o][:],
                                op0=ALU.add, op1=ALU.add,
                            )
                            x1.append(xo)
                            sq = pC.tile([128, CH], BF16, tag=f"sq2_{o}",
                                         name=f"sq2_{o}")
                            nc.scalar.activation(sq[:], xo[:], AF.Square)
                            sq2.append(sq)
                        sum2 = psC.tile([128, CH], F32, tag="s2", name="s2")
                        for o in range(KP):
                            nc.tensor.matmul(sum2[:], ones_r[:],
                                             x1[o][:].bitcast(F32R),
                                             start=(o == 0),
                                             stop=(o == KP - 1))
                        msq2 = psC.tile([128, CH], F32, tag="q2", name="q2")
                        for o in range(KP):
                            nc.tensor.matmul(msq2[:], ones_bf[:], sq2[o][:],
                                             start=(o == 0),
                                             stop=(o == KP - 1))
                        mu2 = pC.tile([128, CH], F32, tag="mu2", name="mu2",
                                      bufs=2)
                        nc.vector.tensor_copy(mu2[:], sum2[:])
                        tv = pC.tile([128, CH], F32, tag="tv", name="tv",
                                     bufs=2)
                        nc.vector.tensor_mul(tv[:], mu2[:], mu2[:])
                        vr = pC.tile([128, CH], F32, tag="vr", name="vr",
                                     bufs=2)
                        nc.vector.tensor_tensor(vr[:], msq2[:], tv[:],
                                                ALU.subtract)
                        sd = pC.tile([128, CH], F32, tag="sd", name="sd",
                                     bufs=2)
                        nc.scalar.activation(sd[:], vr[:], AF.Sqrt,
                                             bias=eps_t[:])
                        rho2 = pC.tile([128, CH], F32, tag="rho2", name="rho2",
                                       bufs=2)
                        nc.vector.reciprocal(rho2[:], sd[:])
                        h2 = []
                        for ct in range(KP):
                            ht = pC.tile([128, CH], F32, tag="h2t", name="h2t",
                                         bufs=2)
                            nc.vector.tensor_tensor(ht[:], x1[ct][:], mu2[:],
                                                    ALU.subtract)
                            h = pC.tile([128, CH], BF16, tag=f"h2_{ct}",
                                        name=f"h2_{ct}", bufs=2)
                            nc.vector.tensor_mul(h[:], ht[:], rho2[:])
                            h2.append(h)
                        if prev is not None:
                            emit_mlp(*prev)
                        prev = (h2, x1, cs)
                    emit_mlp(*prev)

            _loop_ctx.close()

    nc.compile()
    return nc


def _phase_a(nc, tc, bass, xT_d, msq_row_d, rho_row_d, nmr_row_d, qk_ext,
             qksum_bc, qkb_bc, ones_r, eps_t, e_ps):
    with (
        tc.tile_pool(name="pA", bufs=1) as pA,
        tc.tile_pool(name="psA", bufs=1, space="PSUM") as psA,
    ):
        rho_t = pA.tile([128, TS * NCH], F32, tag="rho_t", name="rho_t")
        nmr_t = pA.tile([128, TS * NCH], F32, tag="nmr_t", name="nmr_t")
        mu_t = pA.tile([128, TS * NCH], F32, tag="mu_t", name="mu_t")
        prev_xqc = None

        def emit_energy(xqc, cc):
            for ts in range(TS):
                for qh in range(2):
                    nc.tensor.matmul(
                        e_ps[qh][:],
                        xqc[ts][:, qh * 128:(qh + 1) * 128],
                        xqc[ts][:],
                        start=(cc == 0 and ts == 0),
                        stop=(cc == NCH - 1 and ts == TS - 1),
                        skip_group_check=True,
                    )

        for c in range(NCH):
            cs = slice(c * CH, (c + 1) * CH)
            xt = []
            for p in range(KP):
                t = pA.tile([128, CH], F32, tag=f"xt{p}", name=f"xt{p}",
                            bufs=2)
                nc.sync.dma_start(t[:], xT_d[p * 128:(p + 1) * 128, cs])
                xt.append(t)
            # msq directly in token-part layout: tiny column matmuls
            # (stationary = x^2 sub-tile, moving = ones/P column)
            msq_ps4 = [psA.tile([128, 2], F32, tag=f"msqp{ts}",
                                name=f"msqp{ts}", bufs=1)
                       for ts in range(TS)]
            for p in range(KP):
                sq = pA.tile([128, CH], F32R, tag=f"sq{p % 2}",
                             name=f"sq{p % 2}", bufs=2)
                nc.scalar.activation(sq[:], xt[p][:], AF.Square)
                for ts in range(TS):
                    nc.tensor.matmul(
                        msq_ps4[ts][:],
                        sq[:, ts * 128:(ts + 1) * 128],
                        ones_col[:],
                        start=(p == 0), stop=(p == KP - 1),
                        skip_group_check=True,
                    )
            # x_q (token-part) + mean ride-along; per-ts rho + correction
            xq_c = []
            for ts in range(TS):
                xq_ps = psA.tile([128, P4 + 2], F32, tag="xq", name="xq_ps",
                                 bufs=2)
                for p in range(KP):
                    nc.tensor.matmul(
                        xq_ps[:],
                        xt[p][:].bitcast(F32R)[:, ts * 128:(ts + 1) * 128],
                        qk_ext[p][:],
                        start=(p == 0), stop=(p == KP - 1),
                    )
                col = c * TS + ts
                mu_c = mu_t[:, col:col + 1]
                nc.vector.tensor_copy(mu_c, xq_ps[:, P4:P4 + 1])
                tmp1 = pA.tile([128, 1], F32, tag="tmp1", name="tmp1", bufs=2)
                nc.vector.tensor_mul(tmp1[:], mu_c, mu_c)
                var1 = pA.tile([128, 1], F32, tag="var1", name="var1", bufs=2)
                nc.vector.tensor_tensor(var1[:], msq_ps4[ts][:, 0:1],
                                        tmp1[:], ALU.subtract)
                sd1 = pA.tile([128, 1], F32, tag="sd1", name="sd1", bufs=2)
                nc.scalar.activation(sd1[:], var1[:], AF.Sqrt, bias=eps_t[:])
                rho_c = rho_t[:, col:col + 1]
                nc.vector.reciprocal(rho_c, sd1[:])
                nmr_c = nmr_t[:, col:col + 1]
                nc.vector.scalar_tensor_tensor(
                    nmr_c, mu_c, -1.0, rho_c, op0=ALU.mult, op1=ALU.mult)
                qtmp = pA.tile([128, P4], F32, tag="qtmp", name="qtmp",
                               bufs=2)
                nc.vector.scalar_tensor_tensor(
                    qtmp[:], qksum_bc[:], nmr_c, qkb_bc[:],
                    op0=ALU.mult, op1=ALU.add,
                )
                xc = pA.tile([128, P4], F32R, tag=f"xqc{ts}", name=f"xqc{ts}",
                             bufs=2)
                nc.vector.scalar_tensor_tensor(
                    xc[:], xq_ps[:, :P4], rho_c, qtmp[:],
                    op0=ALU.mult, op1=ALU.add,
                )
                xq_c.append(xc)
            if prev_xqc is not None:
                emit_energy(prev_xqc, c - 1)
            prev_xqc = xq_c
        emit_energy(prev_xqc, NCH - 1)
        # per-token rows -> DRAM (token-part -> row layout)
        for src, dst in ((rho_t, rho_row_d), (nmr_t, nmr_row_d)):
            d = dst[:]
            nc.sync.dma_start(
                bass.AP(tensor=d.tensor, offset=d.offset,
                        ap=[[1, 128], [128, TS * NCH]]),
                src[:],
            )


_CACHE = {}


def _get_nc(loop_R=1):
    if loop_R not in _CACHE:
        _CACHE[loop_R] = _build(loop_R)
    return _CACHE[loop_R]


def _prep_base(inputs):
    f32 = lambda k: np.asarray(inputs[k], np.float32)
    bf16 = ml_dtypes.bfloat16
    g1, b1 = f32("ln1_g"), f32("ln1_b")
    g2, b2 = f32("ln2_g"), f32("ln2_b")
    qk_w = f32("qk_w") * g1[None, :]
    v_w = f32("v_w") * g1[None, :]
    m1_w = f32("m1_w") * g2[None, :]
    return {
        "qk_wT": np.ascontiguousarray(qk_w.T),
        "qksum": np.ascontiguousarray(qk_w.sum(axis=1)),
        "qkb": np.ascontiguousarray(f32("qk_w") @ b1),
        "v_w": np.ascontiguousarray(v_w).astype(bf16),
        "v_b": np.ascontiguousarray(
            f32("v_b") + f32("v_w") @ b1).astype(bf16),
        "t1_wT": np.ascontiguousarray(f32("t1_w").T),
        "t1_b": np.ascontiguousarray(f32("t1_b")),
        "t2_wT": np.ascontiguousarray(f32("t2_w").T),
        "m1_wT": np.ascontiguousarray(m1_w.T).astype(bf16),
        "m1_b": np.ascontiguousarray(f32("m1_b") + f32("m1_w") @ b2),
        "m2_wT": np.ascontiguousarray(f32("m2_w").T).astype(bf16),
        "m2_b": np.ascontiguousarray(f32("m2_b")),
    }


def kernel(**inputs):
    return _kernel_impl(inputs, loop_R=1)


def _kernel_impl(inputs, loop_R=1):
    x = np.ascontiguousarray(np.asarray(inputs["x"], np.float32))
    assert x.shape == (B, N, P), x.shape
    nc = _get_nc(loop_R)
    base = _prep_base(inputs)
    in_maps = []
    for b in range(B):
        m = dict(base)
        m["xT"] = np.ascontiguousarray(x[b].T)
        in_maps.append(m)
    res = bass_utils.run_bass_kernel_spmd(nc, in_maps, core_ids=list(range(B)))
    out = np.empty((B, N, P), np.float32)
    for b in range(B):
        out[b] = res.results[b]["outT"].T
    return out


if __name__ == "__main__":
    import sys
    import time

    sys.path.insert(0, "/root/problem")
    import reference as refmod

    inputs = {k: np.asarray(v) for k, v in refmod.setup_inputs().items()}
    t0 = time.time()
    got = kernel(**inputs)
    print(f"kernel() took {time.time() - t0:.1f}s (incl compile)")
    exp = np.asarray(refmod.reference(**inputs))
    err = np.abs(got - exp)
    l2 = np.linalg.norm(got - exp) / np.linalg.norm(exp)
    print(f"absmax={err.max():.3e} L2rel={l2:.3e}")
